# revision 21
# baseline (speedup 1.0000x reference)
"""MultiHeadedAttention Trainium2 Bass kernel.

Reference (per batch element b, full shapes B=8, S=1024, D=512, H=8, DK=64):
    Q = x_q @ Wq + bq ; K = x_k @ Wk + bk ; V = x_v @ Wv + bv   (per-head split)
    S = Q K^T / sqrt(DK);  S masked where mask==0 -> -inf
    P = softmax(S); P zeroed where mask==0
    Y = (P V, heads concat) @ Wo + bo

Sharding: pure data parallel over batch — core c computes batch element c.
No collectives. Host transposes x inputs so the kernel needs no on-chip
input transposes, precomputes the additive exp-space mask bias, and
rounds matmul operands to bf16 (PSUM accumulation stays fp32; measured
bf16 matmul streams ~15% faster than f32r on HW and halves HBM+SBUF).

Engine assignment: ACT does exp + the V/output PSUM->SBUF moves (Copy
and Exp live in the same activation table, so no table swaps; y-store
DMAs issue on ACT's queue right after the copy — issuing them after a
DVE copy head-of-line-blocks the exp stream); DVE does the projection
bias+move, reciprocal and normalize multiply (ACT is the saturated
engine in steady state); Pool (gpsimd) broadcasts the softmax
reciprocal across partitions.

All tile pools and tiles are allocated ONCE, outside the benchmark
loop: iterations then overlap through per-tile dependencies instead of
a pool-lifetime barrier, so the next iteration's input DMA streams in
under the current iteration's attention phase. Input DMAs issue on
SP's queue; y stores issue on ACT's queue (separate FIFO, so the next
iteration's loads don't queue behind this iteration's stores).

Per-core layout (bf16 matmul operands; PSUM accumulates f32):
  xT        [in=512, S]  (host-transposed)
  QT, KT    [feat, S]   psum[out128, q512] += Wq[in128, out128].T @ xT[in128, q512]
                        bias folded into the ACT Identity that moves PSUM->SBUF
  V natural [S, feat]   psum[row128, f512] += xT_v[in128, row128].T @ Wv[in128, f512]
                        (+ ones-row x bv outer product), stored interleaved as
                        v_aug[row128, head, 65] with a ones column per head
                        (softmax denominator for free); one ACT copy per row tile
  S^T       [k128, q512] = KT_h[d64, k128].T @ QT_h[d64, q512]
  P^T       = Exp(S^T/8 + maskbias_k)      (ACT, one call per [128,1024])
  (PV)^T+den[65, q512]  += v_aug_h[k128, 65].T @ P^T[k128, q512]  (row 64 = denom)
            software-pipelined: pair t-1's four PV chains run in two
            16-link bursts inside pair t's scores+exp stream
  norm      rec = 1/den (DVE), rbs = bcast rec over 64 partitions (Pool),
            at_pair[t][h%2*64 :+64, q] = (PV)^T * rbs (DVE; write may shift
            base partition by 64 for odd heads -> K=128 below)
  Y natural [q128, 512] += at_pair[t][:, q128].T @ Wo[feat128, out512] (+ bo)

PSUM (8 banks): b512 tag ([128,512] x4: proj chains, PV chains, out-proj)
+ scores [128,1024] x2 = 4 banks.
"""

import numpy as np

B, S, D, H = 8, 1024, 512, 8
DK = D // H  # 64
P = 128
KI = D // P  # 4 in-feature tiles
RT = S // P  # 8 row tiles
QC = S // 512  # 2 q chunks of 512
HP = H // 2  # 4 head pairs
MASK_NEG = -30000.0  # exp(-30000) == 0.0 in f32

_CACHED = {}


def _build_nc(loop_reps=None):
    import concourse.mybir as mybir
    import concourse.tile as tile
    from concourse import bacc

    f32 = mybir.dt.float32
    bf16 = mybir.dt.bfloat16
    EXP = mybir.ActivationFunctionType.Exp
    CPY = mybir.ActivationFunctionType.Copy
    IDN = mybir.ActivationFunctionType.Identity
    ISCALE = 1.0 / float(np.sqrt(DK))

    nc = bacc.Bacc("TRN2")

    xqT_d = nc.dram_tensor("xqT", (KI, P, S), bf16, kind="ExternalInput")
    xkT_d = nc.dram_tensor("xkT", (KI, P, S), bf16, kind="ExternalInput")
    xvT_d = nc.dram_tensor("xvT", (KI, P, S), bf16, kind="ExternalInput")
    maskb_d = nc.dram_tensor("maskb", (P, RT), f32, kind="ExternalInput")
    wq_d = nc.dram_tensor("wq", (KI, P, D), bf16, kind="ExternalInput")
    wk_d = nc.dram_tensor("wk", (KI, P, D), bf16, kind="ExternalInput")
    wv_d = nc.dram_tensor("wv", (KI, P, D), bf16, kind="ExternalInput")
    wo_d = nc.dram_tensor("wo", (KI, P, D), bf16, kind="ExternalInput")
    bq_d = nc.dram_tensor("bq", (P, KI), f32, kind="ExternalInput")
    bk_d = nc.dram_tensor("bk", (P, KI), f32, kind="ExternalInput")
    bv_d = nc.dram_tensor("bv", (1, D), bf16, kind="ExternalInput")
    bo_d = nc.dram_tensor("bo", (1, D), bf16, kind="ExternalInput")
    y_d = nc.dram_tensor("y", (RT, P, D), f32, kind="ExternalOutput")

    with tile.TileContext(nc) as tc, nc.allow_low_precision(
        reason="bf16 matmul operands; accumulation stays fp32 in PSUM"
    ):
        from contextlib import ExitStack

        with ExitStack() as ctx:
            const = ctx.enter_context(tc.tile_pool(name="const", bufs=1))
            persist = ctx.enter_context(tc.tile_pool(name="persist", bufs=1))
            xt_pool = ctx.enter_context(tc.tile_pool(name="xt", bufs=1))
            pt_pool = ctx.enter_context(tc.tile_pool(name="pt", bufs=34))
            rec_pool = ctx.enter_context(tc.tile_pool(name="rec", bufs=4))
            rbs_pool = ctx.enter_context(tc.tile_pool(name="rbs", bufs=4))
            y_pool = ctx.enter_context(tc.tile_pool(name="y", bufs=3))
            b512_ps = ctx.enter_context(tc.tile_pool(name="b512", bufs=4, space="PSUM"))
            at_ps = ctx.enter_context(tc.tile_pool(name="spsum", bufs=2, space="PSUM"))

            wq = [const.tile([P, D], bf16, name=f"wq{i}", tag=f"wq{i}") for i in range(KI)]
            wk = [const.tile([P, D], bf16, name=f"wk{i}", tag=f"wk{i}") for i in range(KI)]
            wv = [const.tile([P, D], bf16, name=f"wv{i}", tag=f"wv{i}") for i in range(KI)]
            wo = [const.tile([P, D], bf16, name=f"wo{i}", tag=f"wo{i}") for i in range(KI)]
            bq_t = const.tile([P, KI], f32, name="bq_t", tag="bq")
            bk_t = const.tile([P, KI], f32, name="bk_t", tag="bk")
            bv_t = const.tile([1, D], bf16, name="bv_t", tag="bv")
            bo_t = const.tile([1, D], bf16, name="bo_t", tag="bo")
            maskb = const.tile([P, RT], f32, name="maskb", tag="maskb")
            ones_t = const.tile([1, P], bf16, name="ones_t", tag="ones")
            nc.gpsimd.memset(ones_t[:], 1.0)

            # persistent intermediates
            qt = [persist.tile([P, S], bf16, name=f"qt{i}", tag=f"qt{i}") for i in range(KI)]
            kt_ = [persist.tile([P, S], bf16, name=f"kt{i}", tag=f"kt{i}") for i in range(KI)]
            v_aug = [persist.tile([P, H, DK + 1], bf16, name=f"va{i}", tag=f"va{i}") for i in range(RT)]
            # head-pair attention outputs: pair t rows 0:64 = head 2t,
            # rows 64:128 = head 2t+1 => feature rows 128t..128t+127
            at = [persist.tile([P, S], bf16, name=f"at{i}", tag=f"at{i}") for i in range(HP)]
            xqT = [xt_pool.tile([P, S], bf16, name=f"xq{i}", tag=f"xq{i}") for i in range(KI)]
            xkT = [xt_pool.tile([P, S], bf16, name=f"xk{i}", tag=f"xk{i}") for i in range(KI)]
            xvT = [xt_pool.tile([P, S], bf16, name=f"xv{i}", tag=f"xv{i}") for i in range(KI)]

            # ones columns of v_aug are never overwritten by the loop body
            # (the V copy writes [:, :, 0:DK] only), so set them once.
            for rt in range(RT):
                nc.gpsimd.memset(v_aug[rt][:, :, DK : DK + 1], 1.0)

            # pair 3's P^T tiles are persistent: in the benchmark loop
            # its PV chains + the output projection are software-pipelined
            # into the NEXT iteration's pair-0 scores stream (prologue /
            # shifted steady-state body / epilogue), so these tiles carry
            # a value across the loop boundary.
            pts3 = [
                [persist.tile([P, S], bf16, name=f"p3_{s}_{k}", tag=f"p3_{s}_{k}") for k in range(RT)]
                for s in range(2)
            ]

            def dma_part():
                # DMA in consumption order (queue is FIFO): q-path first so
                # the first projection can start after ~1.5MB, then k-path,
                # v-path, output weights. All input loads on SP's queue —
                # the next iteration's loads start as soon as each target
                # tile's last read of this iteration retires.
                for i in range(KI):
                    nc.sync.dma_start(wq[i][:], wq_d[i])
                    nc.sync.dma_start(xqT[i][:], xqT_d[i])
                nc.sync.dma_start(bq_t[:], bq_d[:])
                for i in range(KI):
                    nc.sync.dma_start(wk[i][:], wk_d[i])
                    nc.sync.dma_start(xkT[i][:], xkT_d[i])
                nc.sync.dma_start(bk_t[:], bk_d[:])
                nc.sync.dma_start(maskb[:], maskb_d[:])
                for i in range(KI):
                    nc.sync.dma_start(wv[i][:], wv_d[i])
                    nc.sync.dma_start(xvT[i][:], xvT_d[i])
                nc.sync.dma_start(bv_t[:], bv_d[:])
                for i in range(KI):
                    nc.sync.dma_start(wo[i][:], wo_d[i])
                nc.sync.dma_start(bo_t[:], bo_d[:])

            def qkt_proj(os_=tuple(range(KI))):
                # QT / KT projections; DVE moves PSUM->SBUF, adding the
                # per-partition bias during the copy.
                for w, x, bias, dst in ((wq, xqT, bq_t, qt), (wk, xkT, bk_t, kt_)):
                    for o in os_:
                        for qc in range(QC):
                            ps = b512_ps.tile([P, 512], f32, name="b512", tag="b512")
                            for ki in range(KI):
                                nc.tensor.matmul(
                                    ps[:],
                                    w[ki][:, o * P : (o + 1) * P],
                                    x[ki][:, qc * 512 : (qc + 1) * 512],
                                    start=(ki == 0),
                                    stop=(ki == KI - 1),
                                )
                            nc.vector.tensor_scalar_add(
                                dst[o][:, qc * 512 : (qc + 1) * 512],
                                ps[:],
                                bias[:, o : o + 1],
                            )

            def v_proj():
                # V natural -> v_aug (interleaved heads; ones columns are
                # set once outside the loop)
                for rt in range(RT):
                    ps = b512_ps.tile([P, 512], f32, name="b512", tag="b512")
                    for ki in range(KI):
                        nc.tensor.matmul(
                            ps[:],
                            xvT[ki][:, rt * P : (rt + 1) * P],
                            wv[ki][:],
                            start=(ki == 0),
                            stop=False,
                        )
                    nc.tensor.matmul(
                        ps[:],
                        ones_t[0:1, 0:P],
                        bv_t[0:1, :],
                        start=False,
                        stop=True,
                    )
                    nc.vector.tensor_copy(
                        v_aug[rt][:, :, 0:DK],
                        ps[:].rearrange("p (h d) -> p h d", h=H),
                    )

            def alloc_opsv():
                return [
                    b512_ps.tile([P, 512], f32, name="ops", tag="b512")
                    for _ in range(4)
                ]

            def new_pts():
                return [
                    [pt_pool.tile([P, S], bf16, name="pt", tag="pt") for _ in range(RT)]
                    for _ in range(2)
                ]

            def pv_link(tp, ptsp, opsv, c, kt):
                sub, qc = divmod(c, QC)
                nc.tensor.matmul(
                    opsv[c][0 : DK + 1, :],
                    v_aug[kt][:, 2 * tp + sub, 0 : DK + 1],
                    ptsp[sub][kt][:, qc * 512 : (qc + 1) * 512],
                    start=(kt == 0),
                    stop=(kt == RT - 1),
                )

            def norms(tp, opsv):
                for c in range(4):
                    sub, qc = divmod(c, QC)
                    ops = opsv[c]
                    rec = rec_pool.tile([1, 512], f32, name="rec", tag="rec")
                    nc.vector.reciprocal(rec[0:1, :], ops[DK : DK + 1, :])
                    rbs = rbs_pool.tile([DK, 512], f32, name="rbs", tag="rbs")
                    nc.gpsimd.partition_broadcast(rbs[:], rec[0:1, :])
                    nc.vector.tensor_mul(
                        at[tp][sub * DK : (sub + 1) * DK, qc * 512 : (qc + 1) * 512],
                        ops[0:DK, :],
                        rbs[:],
                    )

            def stream(t, pts, pv=None):
                # Pair t's scores+exp stream (keeps ACT saturated). pv =
                # (tp, ptsp, opsv): the previous pair's four PV chains
                # (sub x qc) slot into PE's slack in two 16-link bursts
                # (after score blocks 8 and 16). Finer interleaving
                # alternates the PE stationary between scores and PV every
                # few matmuls, which measures ~30-60us slower; a single
                # 32-link burst starves ACT's exp stream.
                links = 0
                for kt in range(RT):
                    for sub in range(2):
                        off = sub * DK
                        sps = at_ps.tile([P, S], f32, name="sps", tag="sps")
                        for qc in range(QC):
                            nc.tensor.matmul(
                                sps[:, qc * 512 : (qc + 1) * 512],
                                kt_[t][off : off + DK, kt * P : (kt + 1) * P],
                                qt[t][off : off + DK, qc * 512 : (qc + 1) * 512],
                                start=True,
                                stop=True,
                            )
                        nc.scalar.activation(
                            pts[sub][kt][:],
                            sps[:],
                            EXP,
                            bias=maskb[:, kt : kt + 1],
                            scale=ISCALE,
                        )
                        if pv is not None:
                            blocks_done = kt * 2 + sub + 1
                            target = min((blocks_done * 2 // 16) * 16, 32)
                            tp, ptsp, opsv = pv
                            while links < target:
                                pv_link(tp, ptsp, opsv, links % 4, links // 4)
                                links += 1

            def drain(tp, ptsp):
                opsv = alloc_opsv()
                for kt in range(RT):
                    for c in range(4):
                        pv_link(tp, ptsp, opsv, c, kt)
                norms(tp, opsv)

            def out_part():
                # Output projection: contraction K=128 over head pairs.
                # Two chains interleaved: alternating accumulate chains
                # hides the PSUM read-modify-write dependency between
                # consecutive links of one chain. y stores issue on ACT's
                # queue (ACT produced yt right before; SP's queue stays
                # clear for the next iteration's loads).
                for rt0 in range(0, RT, 2):
                    ypsa = b512_ps.tile([P, D], f32, name="b512", tag="b512")
                    ypsb = b512_ps.tile([P, D], f32, name="b512", tag="b512")
                    for t in range(HP):
                        for yps, rt in ((ypsa, rt0), (ypsb, rt0 + 1)):
                            nc.tensor.matmul(
                                yps[:],
                                at[t][:, rt * P : (rt + 1) * P],
                                wo[t][:],
                                start=(t == 0),
                                stop=False,
                            )
                    for yps, rt in ((ypsa, rt0), (ypsb, rt0 + 1)):
                        nc.tensor.matmul(
                            yps[:],
                            ones_t[0:1, 0:P],
                            bo_t[0:1, :],
                            start=False,
                            stop=True,
                        )
                    for yps, rt in ((ypsa, rt0), (ypsb, rt0 + 1)):
                        yt = y_pool.tile([P, D], f32, name="yt", tag="yt")
                        nc.vector.tensor_copy(yt[:], yps[:])
                        nc.sync.dma_start(y_d[rt], yt[:])

            def attn_unshifted():
                # pairs 0..3 with each pair's PV pipelined into the next
                # pair's stream; pair 3's PV is left pending (drain later)
                prev = None
                for t in range(HP):
                    pts = pts3 if t == HP - 1 else new_pts()
                    if prev is None:
                        stream(t, pts)
                    else:
                        tp, ptsp = prev
                        opsv = alloc_opsv()
                        stream(t, pts, pv=(tp, ptsp, opsv))
                        norms(tp, opsv)
                    prev = (t, pts)

            def emit_single():
                dma_part()
                qkt_proj()
                v_proj()
                attn_unshifted()
                drain(3, pts3)
                out_part()

            def emit_body():
                # Shifted steady state: the previous iteration's pair-3 PV
                # chains fill PE's slack inside THIS iteration's pair-0
                # stream (which otherwise has no PV filler), then its
                # normalize + output projection run under pair-0/1 exps.
                # V-projection is deferred until the old PV chains have
                # consumed v_aug, keeping per-iteration semantics exact.
                dma_part()
                qkt_proj()
                opsv_d = alloc_opsv()
                pts0 = new_pts()
                stream(0, pts0, pv=(3, pts3, opsv_d))
                norms(3, opsv_d)
                out_part()
                v_proj()
                prev = (0, pts0)
                for t in range(1, HP):
                    pts = pts3 if t == HP - 1 else new_pts()
                    tp, ptsp = prev
                    opsv = alloc_opsv()
                    stream(t, pts, pv=(tp, ptsp, opsv))
                    norms(tp, opsv)
                    prev = (t, pts)

            if loop_reps is None:
                emit_single()
            else:
                # benchmark variant: prologue fills the pipeline, For_i
                # repeats the shifted steady-state body, epilogue drains.
                dma_part()
                qkt_proj()
                v_proj()
                attn_unshifted()
                ET = mybir.EngineType
                with tc.For_i(
                    0,
                    loop_reps,
                    1,
                    hint_engines=(ET.PE, ET.Activation, ET.DVE, ET.SP, ET.Pool),
                ):
                    emit_body()
                drain(3, pts3)
                out_part()

    nc.compile()
    return nc


def get_nc(loop_reps=None):
    key = ("nc", loop_reps)
    if key not in _CACHED:
        _CACHED[key] = _build_nc(loop_reps)
    return _CACHED[key]


def make_in_maps(query, key, value, mask, Wq, bq, Wk, bk, Wv, bv, Wo, bo):
    """Shard full inputs into per-core input maps (host-side numpy)."""
    import ml_dtypes

    f = np.float32
    bf = ml_dtypes.bfloat16
    query = np.asarray(query, f)
    key = np.asarray(key, f)
    value = np.asarray(value, f)
    mask = np.asarray(mask)

    def wtiles(W):
        return np.ascontiguousarray(
            np.asarray(W, f).reshape(KI, P, D).astype(bf)
        )

    wq_t, wk_t, wv_t, wo_t = wtiles(Wq), wtiles(Wk), wtiles(Wv), wtiles(Wo)
    bq_t = np.ascontiguousarray(np.asarray(bq, f).reshape(KI, P).T)
    bk_t = np.ascontiguousarray(np.asarray(bk, f).reshape(KI, P).T)
    bv_t = np.ascontiguousarray(np.asarray(bv, f).reshape(1, D).astype(bf))
    bo_t = np.ascontiguousarray(np.asarray(bo, f).reshape(1, D).astype(bf))

    in_maps = []
    for c in range(B):
        xqT = np.ascontiguousarray(query[c].T).reshape(KI, P, S).astype(bf)
        xkT = np.ascontiguousarray(key[c].T).reshape(KI, P, S).astype(bf)
        xvT = np.ascontiguousarray(value[c].T).reshape(KI, P, S).astype(bf)
        mb = np.where(mask[c, 0] == 0, f(MASK_NEG), f(0.0)).astype(f)
        mb = np.ascontiguousarray(mb.reshape(RT, P).T)
        in_maps.append(
            {
                "xqT": xqT,
                "xkT": xkT,
                "xvT": xvT,
                "maskb": mb,
                "wq": wq_t,
                "wk": wk_t,
                "wv": wv_t,
                "wo": wo_t,
                "bq": bq_t,
                "bk": bk_t,
                "bv": bv_t,
                "bo": bo_t,
            }
        )
    return in_maps


def kernel(**inputs):
    from concourse.bass_utils import run_bass_kernel_spmd

    nc = get_nc()
    in_maps = make_in_maps(**inputs)
    res = run_bass_kernel_spmd(nc, in_maps, core_ids=list(range(B)))
    out = np.stack([res.results[c]["y"].reshape(S, D) for c in range(B)])
    return out.astype(np.float32)
